# revision 6
# baseline (speedup 1.0000x reference)
"""GCN layers (3x GCNConv + PReLU + residual + BatchNorm) on 8 Trainium2 cores.

Full-input contract: kernel(**inputs) takes unsharded numpy arrays and
returns the full [50000, 64] float32 output.

Sharding: nodes are dst-sharded 8 ways (6250/core). Per layer, per core:
  1. own-shard dense matmul xw_c = h_c @ W[i] (PE, h kept transposed
     [64, nodes] in SBUF) -> AllGather the full xw table [50000, 64] f32
     into every core's DRAM.
  2. gather xw[src] for the core's ~125k edges with dma_gather. int16
     index limit (32767) is handled by splitting edges by src parity:
     idx = src//2 into a [25000, 128]-strided view of the table, even
     pass reads cols 0:64, odd pass cols 64:128.
  3. segment-sum by dst via one-hot matmuls: edges are host-grouped by
     (parity, dst_tile of 128) and padded to 128-edge blocks. Per block
     the DVE builds onehot[e, slot] = ew[e] * (dst_local[e] == slot)
     (one fused tensor_scalar) and the PE accumulates
     aggT[64 feat, 128 dst] += msg.T @ onehot in PSUM per dst tile.
  4. bias/PReLU/residual/BatchNorm run in the transposed layout where
     per-feature params are per-partition scalars; BN batch stats use a
     [64, 2] AllReduce.

The block->dst-tile structure is made identical across cores (SPMD: one
program) by padding each (parity, tile) group to the max block count
over cores with ew=0 edges.
"""

import numpy as np

N_NODES = 50000
D = 64
L = 3
BN_EPS = 1e-5
N_CORES = 8
SH = N_NODES // N_CORES          # 6250 nodes per core
NT = (SH + 127) // 128           # 49 dst tiles per core
SHP = NT * 128                   # 6272 padded
CHUNK_BLOCKS = 64                # 8192 gathered edges per dma_gather call

_CACHE = {}


def _preprocess(edge_src, edge_dst, edge_weight):
    """Bucket edges by (core, parity, dst_tile), pad to uniform blocks.

    Returns (blocks[2, NT] shared block counts, per-core arrays):
      gidx_w  [16, NBLK*8] int16  wrapped gather indices (src // 2)
      ewf     [128, NBLK] f32     edge weights by (lane, block)
      dlocf   [128, NBLK] f32     dst-slot-in-tile by (lane, block)
    """
    src = np.asarray(edge_src).astype(np.int64)
    dst = np.asarray(edge_dst).astype(np.int64)
    ew = np.asarray(edge_weight).astype(np.float32)

    core = dst // SH
    dl = dst - core * SH
    tile_id = dl >> 7
    lane = (dl & 127).astype(np.float32)
    par = src & 1
    gidx = (src >> 1).astype(np.int16)

    grp = par * NT + tile_id                     # [E] in [0, 2*NT)
    key = core * (2 * NT) + grp
    counts = np.bincount(key, minlength=N_CORES * 2 * NT).reshape(
        N_CORES, 2 * NT)
    blocks = np.maximum(1, (counts.max(axis=0) + 127) // 128)  # [2*NT]
    grp_off_blk = np.concatenate([[0], np.cumsum(blocks)[:-1]])
    nblk = int(blocks.sum())
    npos = nblk * 128
    grp_off = grp_off_blk * 128

    per_core = []
    for c in range(N_CORES):
        sel = np.flatnonzero(core == c)
        g = grp[sel]
        order = np.argsort(g, kind="stable")
        es = sel[order]
        gs = g[order]
        start_in_sorted = np.searchsorted(gs, np.arange(2 * NT))
        rank = np.arange(len(gs)) - start_in_sorted[gs]
        pos = grp_off[gs] + rank

        G = np.zeros(npos, np.int16)
        EW = np.zeros(npos, np.float32)
        DL = np.zeros(npos, np.float32)
        G[pos] = gidx[es]
        EW[pos] = ew[es]
        DL[pos] = lane[es]
        per_core.append({
            "gidx": np.ascontiguousarray(G.reshape(nblk * 8, 16).T),
            "ewf": np.ascontiguousarray(EW.reshape(nblk, 128).T),
            "dlocf": np.ascontiguousarray(DL.reshape(nblk, 128).T),
        })
    return blocks.reshape(2, NT), per_core


def _build(blocks, prelu_a):
    """Build the SPMD Bass program. blocks: [2, NT] ints; prelu_a: [L]."""
    import concourse.bacc as bacc
    import concourse.mybir as mybir
    import concourse.tile as tile
    from concourse.alu_op_type import AluOpType

    f32 = mybir.dt.float32
    AF = mybir.ActivationFunctionType
    nblk = int(blocks.sum())
    nblk_pass = [int(blocks[0].sum()), int(blocks[1].sum())]
    # block -> dst tile, in canonical order (pass 0 tiles, pass 1 tiles)
    tob = []
    for p in range(2):
        for t in range(NT):
            tob.extend([t] * int(blocks[p][t]))
    # first/last block index of each (pass, tile)
    first_blk = {}
    last_blk = {}
    b = 0
    for p in range(2):
        for t in range(NT):
            first_blk[(p, t)] = b
            b += int(blocks[p][t])
            last_blk[(p, t)] = b - 1

    nc = bacc.Bacc("TRN2", target_bir_lowering=False, debug=False,
                   num_devices=N_CORES)

    xT = nc.dram_tensor("xT", [D, SHP], f32, kind="ExternalInput")
    Wt = nc.dram_tensor("Wt", [L * D, D], f32, kind="ExternalInput")
    bgb = nc.dram_tensor("bgb", [D, 3 * L], f32, kind="ExternalInput")
    iota_in = nc.dram_tensor("iota", [128, 128], f32, kind="ExternalInput")
    ident_in = nc.dram_tensor("ident", [128, 128], f32, kind="ExternalInput")
    gidx_in = nc.dram_tensor("gidx", [16, nblk * 8], mybir.dt.int16,
                             kind="ExternalInput")
    ewf_in = nc.dram_tensor("ewf", [128, nblk], f32, kind="ExternalInput")
    dlocf_in = nc.dram_tensor("dlocf", [128, nblk], f32,
                              kind="ExternalInput")
    out = nc.dram_tensor("out", [SH, D], f32, kind="ExternalOutput")

    with tile.TileContext(nc) as tc:
        with (
            tc.tile_pool(name="const", bufs=1) as cpool,
            tc.tile_pool(name="small", bufs=2) as spool,
            tc.tile_pool(name="gath", bufs=3) as gpool,
            tc.tile_pool(name="oh", bufs=6) as opool,
            tc.tile_pool(name="xwsb", bufs=3) as xpool,
            tc.tile_pool(name="psagg", bufs=4, space="PSUM") as pspool,
            tc.tile_pool(name="psxw", bufs=2, space="PSUM") as pxpool,
            tc.tile_pool(name="dram", bufs=1, space="DRAM") as dpool,
        ):
            # ---- constants / persistent buffers ----
            h = cpool.tile([D, SHP], f32, tag="h")
            aggE = cpool.tile([D, SHP], f32, tag="aggE")
            aggO = cpool.tile([D, SHP], f32, tag="aggO")
            tmp = cpool.tile([D, SHP], f32, tag="tmp")
            iota_sb = cpool.tile([128, 128], f32, tag="iota")
            ident_sb = cpool.tile([128, 128], f32, tag="ident")
            gidx_sb = cpool.tile([128, nblk * 8], mybir.dt.int16, tag="gidx")
            ewf_sb = cpool.tile([128, nblk], f32, tag="ewf")
            dlocf_sb = cpool.tile([128, nblk], f32, tag="dlocf")
            bgb_sb = cpool.tile([D, 3 * L], f32, tag="bgb")
            stat_sb = cpool.tile([D, 2], f32, tag="stat")
            statr_sb = cpool.tile([D, 2], f32, tag="statr")

            nc.sync.dma_start(h[:], xT[:])
            nc.sync.dma_start(iota_sb[:], iota_in[:])
            nc.sync.dma_start(ident_sb[:], ident_in[:])
            for r in range(8):
                nc.sync.dma_start(gidx_sb[16 * r:16 * (r + 1), :], gidx_in[:])
            nc.sync.dma_start(ewf_sb[:], ewf_in[:])
            nc.sync.dma_start(dlocf_sb[:], dlocf_in[:])
            nc.sync.dma_start(bgb_sb[:], bgb[:])

            ag_in = dpool.tile([SH, D], f32, tag="ag_in")
            table = dpool.tile([N_NODES // 2, 2 * D], f32, tag="table")
            bn_in = dpool.tile([D, 2], f32, tag="bn_in")
            bn_out = dpool.tile([D, 2], f32, tag="bn_out")

            for i in range(L):
                # ---- dense: xw_c = h_c @ W[i], written to ag_in ----
                w_sb = spool.tile([D, D], f32, tag="w_sb")
                nc.sync.dma_start(w_sb[:], Wt[i * D:(i + 1) * D, :])
                for t in range(NT):
                    ps = pxpool.tile([128, D], f32, tag="ps_xw")
                    nc.tensor.matmul(ps[:], h[:, t * 128:(t + 1) * 128],
                                     w_sb[:], start=True, stop=True)
                    xw_sb = xpool.tile([128, D], f32, tag="xw_sb")
                    nc.scalar.activation(xw_sb[:], ps[:], AF.Copy)
                    rows = min(128, SH - t * 128)
                    nc.sync.dma_start(
                        ag_in[t * 128:t * 128 + rows, :], xw_sb[:rows, :])

                nc.gpsimd.collective_compute(
                    "AllGather",
                    mybir.AluOpType.bypass,
                    replica_groups=[list(range(N_CORES))],
                    ins=[ag_in.opt()],
                    outs=[table.opt()],
                )

                # ---- gather + one-hot matmul scatter, two parity passes ----
                for p in range(2):
                    agg_buf = aggE if p == 0 else aggO
                    p0 = 0 if p == 0 else nblk_pass[0]
                    p1 = p0 + nblk_pass[p]
                    cur_t = -1
                    ps_agg = None
                    for c0 in range(p0, p1, CHUNK_BLOCKS):
                        c1 = min(c0 + CHUNK_BLOCKS, p1)
                        cb = c1 - c0
                        g = gpool.tile([128, CHUNK_BLOCKS, D], f32, tag="g")
                        nc.gpsimd.dma_gather(
                            g[:, :cb, :],
                            table[:, p * D:(p + 1) * D],
                            gidx_sb[:, c0 * 8:c1 * 8],
                            cb * 128,
                            cb * 128,
                            D,
                            elem_step=2 * D,
                            single_packet=False,
                        )
                        for blk in range(c0, c1):
                            t = tob[blk]
                            if t != cur_t:
                                ps_agg = pspool.tile([D, 128], f32,
                                                     tag="ps_agg")
                                cur_t = t
                            oh = opool.tile([128, 128], f32, tag="oh")
                            nc.vector.tensor_scalar(
                                oh[:], iota_sb[:],
                                dlocf_sb[:, blk:blk + 1],
                                ewf_sb[:, blk:blk + 1],
                                AluOpType.is_equal, AluOpType.mult)
                            nc.tensor.matmul(
                                ps_agg[:], g[:, blk - c0, :], oh[:],
                                start=(blk == first_blk[(p, t)]),
                                stop=(blk == last_blk[(p, t)]))
                            if blk == last_blk[(p, t)]:
                                nc.scalar.activation(
                                    agg_buf[:, t * 128:(t + 1) * 128],
                                    ps_agg[:], AF.Copy)

                # ---- post: bias, PReLU, residual, BatchNorm ----
                bias_ap = bgb_sb[:, i:i + 1]
                gamma_ap = bgb_sb[:, L + i:L + i + 1]
                beta_ap = bgb_sb[:, 2 * L + i:2 * L + i + 1]
                a_i = float(prelu_a[i])

                # tmp = (aggE + bias) + aggO
                nc.vector.scalar_tensor_tensor(
                    tmp[:], aggE[:], bias_ap, aggO[:],
                    AluOpType.add, AluOpType.add)
                # aggE = relu(tmp) ; aggO = a * min(tmp, 0)
                nc.scalar.activation(aggE[:], tmp[:], AF.Relu)
                nc.vector.tensor_scalar(aggO[:], tmp[:], 0.0, a_i,
                                        AluOpType.min, AluOpType.mult)
                if i > 0:
                    # tmp = prelu ; aggE = tmp + h (residual)
                    nc.vector.tensor_tensor(tmp[:], aggE[:], aggO[:],
                                            AluOpType.add)
                    nc.vector.tensor_tensor(aggE[:], tmp[:], h[:],
                                            AluOpType.add)
                else:
                    nc.vector.tensor_tensor(aggE[:], aggE[:], aggO[:],
                                            AluOpType.add)

                # BN stats over valid nodes
                nc.vector.reduce_sum(stat_sb[:, 0:1], aggE[:, 0:SH],
                                     axis=mybir.AxisListType.X)
                nc.scalar.activation(tmp[:, 0:SH], aggE[:, 0:SH], AF.Square,
                                     accum_out=stat_sb[:, 1:2])
                nc.sync.dma_start(bn_in[:], stat_sb[:])
                nc.gpsimd.collective_compute(
                    "AllReduce",
                    mybir.AluOpType.add,
                    replica_groups=[list(range(N_CORES))],
                    ins=[bn_in.opt()],
                    outs=[bn_out.opt()],
                )
                nc.sync.dma_start(statr_sb[:], bn_out[:])

                mean = spool.tile([D, 1], f32, tag="mean")
                var = spool.tile([D, 1], f32, tag="var")
                scl = spool.tile([D, 1], f32, tag="scl")
                shf = spool.tile([D, 1], f32, tag="shf")
                t0 = spool.tile([D, 1], f32, tag="t0")
                nc.vector.tensor_scalar(mean[:], statr_sb[:, 0:1],
                                        1.0 / N_NODES, None, AluOpType.mult)
                nc.vector.tensor_scalar(var[:], statr_sb[:, 1:2],
                                        1.0 / N_NODES, None, AluOpType.mult)
                nc.vector.tensor_tensor(t0[:], mean[:], mean[:],
                                        AluOpType.mult)
                nc.vector.tensor_tensor(var[:], var[:], t0[:],
                                        AluOpType.subtract)
                nc.vector.tensor_scalar(var[:], var[:], BN_EPS, None,
                                        AluOpType.add)
                nc.scalar.activation(t0[:], var[:], AF.Sqrt)
                nc.vector.reciprocal(scl[:], t0[:])
                nc.vector.tensor_tensor(scl[:], scl[:], gamma_ap,
                                        AluOpType.mult)
                nc.vector.tensor_tensor(t0[:], mean[:], scl[:],
                                        AluOpType.mult)
                nc.vector.scalar_tensor_tensor(shf[:], beta_ap, 0.0, t0[:],
                                               AluOpType.add,
                                               AluOpType.subtract)
                # h = aggE * scl + shf
                nc.vector.tensor_scalar(h[:], aggE[:], scl[:], shf[:],
                                        AluOpType.mult, AluOpType.add)

            # ---- un-transpose h -> out [SH, 64] ----
            for t in range(NT):
                ps = pxpool.tile([128, D], f32, tag="ps_xw")
                nc.tensor.transpose(ps[:], h[:, t * 128:(t + 1) * 128],
                                    ident_sb[:D, :D])
                o_sb = xpool.tile([128, D], f32, tag="o_sb")
                nc.scalar.activation(o_sb[:], ps[:], AF.Copy)
                rows = min(128, SH - t * 128)
                nc.sync.dma_start(out[t * 128:t * 128 + rows, :],
                                  o_sb[:rows, :])

    nc.finalize()
    return nc


_LAST_HW_NS = None


def kernel(x, edge_src, edge_dst, edge_weight, W, b, prelu_a,
           bn_gamma, bn_beta):
    import os

    from concourse import bass_utils

    x = np.asarray(x, np.float32)
    W = np.asarray(W, np.float32)
    b = np.asarray(b, np.float32)
    prelu_a = np.asarray(prelu_a, np.float32)
    bn_gamma = np.asarray(bn_gamma, np.float32)
    bn_beta = np.asarray(bn_beta, np.float32)

    blocks, per_core = _preprocess(edge_src, edge_dst, edge_weight)

    key = (tuple(blocks.ravel().tolist()), tuple(prelu_a.tolist()))
    if key not in _CACHE:
        _CACHE[key] = _build(blocks, prelu_a)
    nc = _CACHE[key]

    # shared inputs
    Wt = W.reshape(L * D, D)
    bgb = np.concatenate([b.T, bn_gamma.T, bn_beta.T], axis=1)  # [64, 9]
    iota = np.tile(np.arange(128, dtype=np.float32), (128, 1))
    ident = np.eye(128, dtype=np.float32)

    xpad = np.zeros((N_CORES, D, SHP), np.float32)
    for c in range(N_CORES):
        xpad[c, :, :SH] = x[c * SH:(c + 1) * SH].T

    in_maps = []
    for c in range(N_CORES):
        in_maps.append({
            "xT": xpad[c],
            "Wt": Wt,
            "bgb": bgb,
            "iota": iota,
            "ident": ident,
            "gidx": per_core[c]["gidx"],
            "ewf": per_core[c]["ewf"],
            "dlocf": per_core[c]["dlocf"],
        })

    kw = {}
    if os.environ.get("GCN_TRACE"):
        kw = {"trace": True, "tmpdir": "/tmp/gcn_trace"}
        os.makedirs("/tmp/gcn_trace", exist_ok=True)
    res = bass_utils.run_bass_kernel_spmd(nc, in_maps, list(range(N_CORES)),
                                          **kw)
    global _LAST_HW_NS
    _LAST_HW_NS = res.exec_time_ns
    out = np.empty((N_NODES, D), np.float32)
    for c in range(N_CORES):
        out[c * SH:(c + 1) * SH] = res.results[c]["out"]
    return out
